# revision 1
# baseline (speedup 1.0000x reference)
"""Distributed GCN (3x GCNConv + MLP classifier) on 8 Trainium2 NeuronCores.

Strategy (graph/data parallel, dst-partitioned):
  - nodes are partitioned into 8 contiguous chunks (one per core), padded to a
    multiple of 128; edges are assigned to the core owning their dst node and
    bucketed by (dst block of 128, quarter of 32) with self-loop edges added.
  - per layer: each core computes z = x @ W(folded) for its nodes, scales rows
    by deg_inv_sqrt, AllGathers the z-table (bf16, 256B row stride), then for
    each 128-edge tile dma_gathers z[src] rows and accumulates
    agg[dst] += onehot(dst_rel).T @ z_rows on the TensorEngine (PSUM f32).
  - BatchNorm (eval) and LayerNorm gains are folded into the weights on host.
All heavy math runs on device; the host only preprocesses integer graph
structure, folds parameters, and slices/concatenates per-core arrays.
"""
import sys
import os

for _p in ("/opt/trn_rl_repo",):
    if _p not in sys.path:
        sys.path.insert(0, _p)

import numpy as np
import ml_dtypes

import concourse.bass as bass
import concourse.bacc as bacc
import concourse.tile as tile
import concourse.mybir as mybir
from concourse import bass_utils

BF16 = ml_dtypes.bfloat16
F32 = mybir.dt.float32
BF = mybir.dt.bfloat16
EPS = 1e-5
NC = 8
P = 128          # partitions / block size
W = 32           # one-hot quarter width
TS = 128         # z-table row stride in bf16 elements (256B, dma_gather req)
MAX_GROUP_IDX = 12000   # dma_gather Q7 scratch limit is ~16k int32 idxs


def _patch_dma_gather():
    """Relax bass's elem_size%256B assert: the Q7 ucode only needs the row
    STRIDE to be a multiple of 256B; the payload can be narrower."""
    import inspect, re, textwrap
    import concourse.bass as cbass
    src = textwrap.dedent(inspect.getsource(cbass.BassGpSimd.dma_gather))
    pat = re.compile(
        r"assert \(\s*elem_size_bytes > 0 and elem_size_bytes % 256 == 0\s*\)"
        r"\s*#[^\n]*", re.S)
    assert pat.search(src), "dma_gather source changed; update patch"
    src = pat.sub("assert elem_size_bytes > 0", src)
    ns = vars(cbass).copy()
    exec(compile(src, "<patched_dma_gather>", "exec"), ns)
    cbass.BassGpSimd.dma_gather = ns["dma_gather"]


_patch_dma_gather()


def _to_bf(a):
    return np.ascontiguousarray(np.asarray(a, np.float32)).astype(BF16)


def _rep(v, width=None):
    """Replicate a 1-D param across 128 partitions -> [128, len]."""
    v = np.asarray(v, np.float32).reshape(1, -1)
    return np.ascontiguousarray(np.repeat(v, P, 0))


def preprocess(x, edge_index, ln_g, ln_b, W1, b1, bn1_g, bn1_b, bn1_m, bn1_v,
               W2, b2, bn2_g, bn2_b, bn2_m, bn2_v, W3, b3, bn3_g, bn3_b, bn3_m,
               bn3_v, fc1_W, fc1_b, lnc_g, lnc_b, fc2_W, fc2_b):
    N, D = x.shape
    E = edge_index.shape[1]
    H1, H2, H3 = W1.shape[1], W2.shape[1], W3.shape[1]
    HC, C = fc1_W.shape[1], fc2_W.shape[1]
    assert N % NC == 0, N
    NPC = N // NC
    NBLK = (NPC + P - 1) // P
    NPAD = NBLK * P
    NTAB = NC * NPAD
    assert NTAB <= 65536
    BIAS = max(0, NTAB - 32768)

    src = np.asarray(edge_index[0], np.int64)
    dst = np.asarray(edge_index[1], np.int64)
    deg = np.bincount(dst, minlength=N).astype(np.float32) + 1.0
    dis = 1.0 / np.sqrt(deg)

    # fold LN gain + BN(eval) into weights; biases:
    #   z1 = x_ln_raw @ W1f + zb1 (per-row const), post-agg bias b1f
    k1 = bn1_g / np.sqrt(bn1_v + EPS)
    W1f = (np.asarray(ln_g)[:, None] * np.asarray(W1)) * k1[None, :]
    zb1 = (np.asarray(ln_b) @ np.asarray(W1)) * k1
    b1f = np.asarray(b1) * k1 + (bn1_b - bn1_m * k1)
    k2 = bn2_g / np.sqrt(bn2_v + EPS)
    W2f = np.asarray(W2) * k2[None, :]
    b2f = np.asarray(b2) * k2 + (bn2_b - bn2_m * k2)
    k3 = bn3_g / np.sqrt(bn3_v + EPS)
    W3f = np.asarray(W3) * k3[None, :]
    b3f = np.asarray(b3) * k3 + (bn3_b - bn3_m * k3)

    # edge lists: original edges + self loops, assigned to dst owner core
    src_all = np.concatenate([src, np.arange(N, dtype=np.int64)])
    dst_all = np.concatenate([dst, np.arange(N, dtype=np.int64)])
    core_of = dst_all // NPC
    dloc = dst_all - core_of * NPC
    srcpad = (src_all // NPC) * NPAD + (src_all % NPC)

    ncell = NBLK * 4
    counts = np.zeros((NC, ncell), np.int64)
    per_core = []
    for c in range(NC):
        m = core_of == c
        s = srcpad[m]
        d = dloc[m]
        o = np.argsort(d, kind="stable")
        s, d = s[o], d[o]
        cell = (d >> 5).astype(np.int64)  # block*4 + quarter
        counts[c] = np.bincount(cell, minlength=ncell)
        per_core.append((s, d, cell))

    T = np.maximum(1, -(-counts.max(0) // P))      # tiles per cell, shared
    tile_off = np.concatenate([[0], np.cumsum(T)]).astype(np.int64)
    ntiles = int(tile_off[-1])

    idx16_list, dstrel_list = [], []
    for c in range(NC):
        s, d, cell = per_core[c]
        start = np.searchsorted(cell, np.arange(ncell))
        pos = np.arange(len(cell)) - start[cell]
        slot = tile_off[cell] * P + pos
        idx_lin = np.zeros(ntiles * P, np.int32)          # pad -> row BIAS
        rel_lin = np.full(ntiles * P, 99.0, np.float32)   # pad -> no match
        idx_lin[slot] = (s - BIAS).astype(np.int32)
        rel_lin[slot] = (d & 31).astype(np.float32)
        assert idx_lin.min() >= -32768 and idx_lin.max() <= 32767
        idx16 = idx_lin.reshape(ntiles * 8, 16).T.astype(np.int16)  # n=(n%16,n//16)
        idx16 = np.tile(idx16, (8, 1))                     # replicate: [128, ntiles*8]
        dstrel = rel_lin.reshape(ntiles, P).T              # [128, ntiles]
        idx16_list.append(np.ascontiguousarray(idx16))
        dstrel_list.append(np.ascontiguousarray(_to_bf(dstrel)))

    # group blocks so one dma_gather stays under the Q7 scratch limit
    groups = []  # list of (b0, nb) block ranges
    b0 = 0
    while b0 < NBLK:
        nb = 0
        while (b0 + nb < NBLK and nb < 8
               and (tile_off[(b0 + nb + 1) * 4] - tile_off[b0 * 4]) * P
               <= MAX_GROUP_IDX):
            nb += 1
        nb = max(nb, 1)
        groups.append((b0, nb))
        b0 += nb

    # per-core node data
    xp_list, disb_list = [], []
    dis_pad = np.ones(NC * NPAD, np.float32)
    for c in range(NC):
        xp = np.zeros((NPAD, D), np.float32)
        xp[:NPC] = np.asarray(x[c * NPC:(c + 1) * NPC], np.float32)
        xp_list.append(xp)
        dis_pad[c * NPAD:c * NPAD + NPC] = dis[c * NPC:(c + 1) * NPC]
        disb = dis_pad[c * NPAD:(c + 1) * NPAD].reshape(NBLK, P).T
        disb_list.append(np.ascontiguousarray(disb))

    iota = np.tile(np.arange(W, dtype=np.float32), (P, 16))
    ident = np.eye(P, dtype=np.float32)

    consts = dict(
        w1=_to_bf(W1f), w2=_to_bf(W2f), w3=_to_bf(W3f),
        fc1w=_to_bf(np.asarray(fc1_W)), fc2w=_to_bf(np.asarray(fc2_W)),
        zb1=_rep(zb1), b1f=_rep(b1f), b2f=_rep(b2f), b3f=_rep(b3f),
        fc1b=_rep(fc1_b), lncg=_rep(lnc_g), lncb=_rep(lnc_b), fc2b=_rep(fc2_b),
        iota=_to_bf(iota), idn=_to_bf(ident),
    )
    in_maps = []
    for c in range(NC):
        m = dict(consts)
        m.update(xp=xp_list[c], disb=disb_list[c], idx16=idx16_list[c],
                 dstrel=dstrel_list[c])
        in_maps.append(m)

    cfg = dict(N=N, D=D, E=E, H1=H1, H2=H2, H3=H3, HC=HC, C=C, NPC=NPC,
               NBLK=NBLK, NPAD=NPAD, NTAB=NTAB, BIAS=BIAS, ntiles=ntiles,
               T=T.tolist(), tile_off=tile_off.tolist(), groups=groups)
    return cfg, in_maps


def build_nc(cfg):
    stop = cfg.get("stop", "")
    D, H1, H2, H3 = cfg["D"], cfg["H1"], cfg["H2"], cfg["H3"]
    HC, C = cfg["HC"], cfg["C"]
    NBLK, NPAD, NTAB, BIAS = cfg["NBLK"], cfg["NPAD"], cfg["NTAB"], cfg["BIAS"]
    ntiles, T, tile_off = cfg["ntiles"], cfg["T"], cfg["tile_off"]
    groups = cfg["groups"]
    KD = D // P      # k-chunks for layer-1 matmul

    nc = bacc.Bacc("TRN2", target_bir_lowering=False, debug=False,
                   num_devices=NC)
    dt = nc.dram_tensor
    ap_xp = dt("xp", [NPAD, D], F32, kind="ExternalInput").ap()
    ap_disb = dt("disb", [P, NBLK], F32, kind="ExternalInput").ap()
    ap_idx16 = dt("idx16", [P, ntiles * 8], mybir.dt.int16, kind="ExternalInput").ap()
    ap_dstrel = dt("dstrel", [P, ntiles], BF, kind="ExternalInput").ap()
    ap_w1 = dt("w1", [D, H1], BF, kind="ExternalInput").ap()
    ap_w2 = dt("w2", [H1, H2], BF, kind="ExternalInput").ap()
    ap_w3 = dt("w3", [H2, H3], BF, kind="ExternalInput").ap()
    ap_fc1w = dt("fc1w", [H3, HC], BF, kind="ExternalInput").ap()
    ap_fc2w = dt("fc2w", [HC, C], BF, kind="ExternalInput").ap()
    reps = {}
    for nm, wd in [("zb1", H1), ("b1f", H1), ("b2f", H2), ("b3f", H3),
                   ("fc1b", HC), ("lncg", HC), ("lncb", HC), ("fc2b", C)]:
        reps[nm] = dt(nm, [P, wd], F32, kind="ExternalInput").ap()
    ap_iota = dt("iota", [P, 16 * W], BF, kind="ExternalInput").ap()
    ap_idn = dt("idn", [P, P], BF, kind="ExternalInput").ap()
    ap_out = dt("out", [NPAD, C], F32, kind="ExternalOutput").ap()

    with tile.TileContext(nc) as tc:
        with (
            tc.tile_pool(name="const", bufs=1) as cp,
            tc.tile_pool(name="xin", bufs=2) as xin,
            tc.tile_pool(name="work", bufs=2) as wk,
            tc.tile_pool(name="small", bufs=3) as sm,
            tc.tile_pool(name="zbuf", bufs=1) as zb,
            tc.tile_pool(name="gath", bufs=2) as gp,
            tc.tile_pool(name="onehot", bufs=2) as op_,
            tc.tile_pool(name="psA", bufs=2, space="PSUM") as psA,
            tc.tile_pool(name="psZ", bufs=2, space="PSUM") as psZ,
            tc.tile_pool(name="psT", bufs=2, space="PSUM") as psT,
            tc.tile_pool(name="dram", bufs=1, space="DRAM") as dram,
        ):
            # ---- constants to SBUF
            def load_const(ap, shape, dtype):
                t = cp.tile(shape, dtype, tag=f"c{ap.tensor.name}",
                            name=f"c{ap.tensor.name}")
                nc.sync.dma_start(t[:], ap)
                return t

            t_w1 = cp.tile([P, KD * H1], BF, tag="w1")
            nc.sync.dma_start(t_w1[:].rearrange("p (k h) -> p k h", h=H1),
                              ap_w1.rearrange("(k p) h -> p k h", p=P))
            t_w2 = load_const(ap_w2, [H1, H2], BF)
            t_w3 = load_const(ap_w3, [H2, H3], BF)
            t_fc1w = load_const(ap_fc1w, [H3, HC], BF)
            t_fc2w = load_const(ap_fc2w, [HC, C], BF)
            t_rep = {}
            for nm in reps:
                t_rep[nm] = load_const(reps[nm], list(reps[nm].shape), F32)
            t_iota = load_const(ap_iota, [P, 16 * W], BF)
            t_idn = load_const(ap_idn, [P, P], BF)
            t_disb = load_const(ap_disb, [P, NBLK], F32)
            t_eps = cp.tile([P, 1], F32, tag="eps")
            nc.vector.memset(t_eps[:], float(EPS))
            t_idx = cp.tile([P, ntiles * 8], mybir.dt.int16, tag="idx")
            nc.sync.dma_start(t_idx[:], ap_idx16)
            t_drel = cp.tile([P, ntiles], BF, tag="drel")
            nc.sync.dma_start(t_drel[:], ap_dstrel)

            z_local = [dram.tile([NPAD, TS], BF, tag=f"zloc{l}",
                                 name=f"zloc{l}") for l in range(3)]
            z_full = [dram.tile([NTAB, TS], BF, tag=f"zfull{l}",
                                name=f"zfull{l}") for l in range(3)]
            HH = [H1, H2, H3]
            zs_buf = [zb.tile([P, NBLK * HH[l]], BF, tag=f"zs{l}",
                              name=f"zs{l}") for l in range(3)]
            out_buf = zb.tile([P, NBLK * C], F32, tag="outb")

            # ============ phase A: LN + z1 per block ============
            for b in range(NBLK):
                xblk = xin.tile([P, D], F32, tag="xblk")
                nc.sync.dma_start(xblk[:], ap_xp[b * P:(b + 1) * P, :])
                ssum = sm.tile([P, 1], F32, tag="ssum")
                nc.vector.reduce_sum(ssum[:], xblk[:], axis=mybir.AxisListType.X,
                                     negate=True)
                negmean = sm.tile([P, 1], F32, tag="negmean")
                nc.vector.tensor_scalar_mul(negmean[:], ssum[:], 1.0 / D)
                sq = wk.tile([P, D], F32, tag="sq")
                sqs = sm.tile([P, 1], F32, tag="sqs")
                nc.scalar.activation(sq[:], xblk[:],
                                     mybir.ActivationFunctionType.Square,
                                     bias=negmean[:], scale=1.0,
                                     accum_out=sqs[:])
                std = sm.tile([P, 1], F32, tag="std")
                nc.scalar.activation(std[:], sqs[:],
                                     mybir.ActivationFunctionType.Sqrt,
                                     bias=t_eps[:], scale=1.0 / D)
                rstd = sm.tile([P, 1], F32, tag="rstd")
                nc.vector.reciprocal(rstd[:], std[:])
                nmr = sm.tile([P, 1], F32, tag="nmr")
                nc.vector.tensor_tensor(nmr[:], negmean[:], rstd[:],
                                        op=mybir.AluOpType.mult)
                xln = wk.tile([P, D], BF, tag="xln")
                nc.vector.tensor_scalar(xln[:], xblk[:], rstd[:], nmr[:],
                                        op0=mybir.AluOpType.mult,
                                        op1=mybir.AluOpType.add)
                # transpose to [D, 128] (two 128-chunks), then z1
                zp = psZ.tile([P, H1], F32, tag="zps")
                for kc in range(KD):
                    tp = psT.tile([P, P], BF, tag="tps")
                    nc.tensor.transpose(tp[:], xln[:, kc * P:(kc + 1) * P],
                                        t_idn[:])
                    xT = wk.tile([P, P], BF, tag="xT")
                    nc.vector.tensor_copy(xT[:], tp[:])
                    nc.tensor.matmul(zp[:], lhsT=xT[:],
                                     rhs=t_w1[:, kc * H1:(kc + 1) * H1],
                                     start=(kc == 0), stop=(kc == KD - 1))
                ztmp = wk.tile([P, H1], F32, tag="ztmp")
                nc.vector.tensor_tensor(ztmp[:], zp[:], t_rep["zb1"][:],
                                        op=mybir.AluOpType.add)
                nc.vector.tensor_scalar_mul(
                    zs_buf[0][:, b * H1:(b + 1) * H1], ztmp[:],
                    t_disb[:, b:b + 1])
            nc.sync.dma_start(
                z_local[0][:].rearrange("(j p) s -> p j s", p=P)[:, :, 0:H1],
                zs_buf[0][:].rearrange("p (j h) -> p j h", h=H1))

            # ============ per-layer edge phases ============
            def edge_layer(l, Fh, Fo, t_wnext, postbias, mode="full"):
                """layer l: gather z_l, aggregate, epilogue -> h; z_{l+1} or
                classifier input written to zs_buf[l+1] (if t_wnext) else
                returns h tiles via classifier()."""
                if cfg.get("no_cc"):
                    # timeline-sim proxy: collectives replaced by equivalent
                    # local DMA traffic (single-core TimelineSim only)
                    for c in range(NC):
                        nc.sync.dma_start(
                            z_full[l][c * NPAD:(c + 1) * NPAD, :], z_local[l][:])
                else:
                    nc.gpsimd.collective_compute(
                        "AllGather", mybir.AluOpType.bypass,
                        replica_groups=[list(range(NC))],
                        ins=[z_local[l][:].opt()], outs=[z_full[l][:].opt()],
                    )
                if mode == "ag":
                    return
                only_gather = mode in ("gather", "gather0")
                for (b0, nb) in groups:
                    t0 = tile_off[b0 * 4]
                    t1 = tile_off[(b0 + nb) * 4]
                    gt = t1 - t0
                    gbuf = gp.tile([P, gt * Fh], BF, tag="gbuf")
                    nc.gpsimd.dma_gather(
                        out_ap=gbuf[:].rearrange("p (n f) -> p n f", f=Fh),
                        in_ap=z_full[l][BIAS:, 0:Fh],
                        idxs_ap=t_idx[:, t0 * 8:t1 * 8],
                        num_idxs=gt * P,
                        num_idxs_reg=gt * P,
                        elem_size=Fh,
                        elem_step=TS,
                        single_packet=False,
                    )
                    sbuf = op_.tile([P, gt * W], BF, tag="sbufS")
                    if mode == "gather0":
                        nc.vector.tensor_copy(out_buf[:, 0:C], gbuf[:, 0:C])
                        continue
                    for s0 in range(0, gt, 16):
                        s1 = min(s0 + 16, gt)
                        dr = t_drel[:, t0 + s0:t0 + s1]
                        dr_b = bass.AP(dr.tensor, dr.offset, dr.ap + [[0, W]])
                        nc.vector.tensor_tensor(
                            out=sbuf[:, s0 * W:s1 * W].rearrange(
                                "p (t w) -> p t w", w=W),
                            in0=t_iota[:, 0:(s1 - s0) * W].rearrange(
                                "p (t w) -> p t w", w=W),
                            in1=dr_b,
                            op=mybir.AluOpType.is_equal)
                    if mode == "gather":
                        nc.vector.tensor_copy(out_buf[:, 0:C], gbuf[:, 0:C])
                        nc.vector.tensor_copy(out_buf[:, C:2 * C],
                                              sbuf[:, 0:C])
                        continue
                    for b in range(b0, b0 + nb):
                        agg = psA.tile([P, Fh], F32, tag="agg")
                        for q in range(4):
                            cell = b * 4 + q
                            nt = T[cell]
                            base = tile_off[cell]
                            for t in range(nt):
                                g = base + t - t0
                                nc.tensor.matmul(
                                    agg[q * W:(q + 1) * W, :],
                                    lhsT=sbuf[:, g * W:(g + 1) * W],
                                    rhs=gbuf[:, g * Fh:(g + 1) * Fh],
                                    start=(t == 0), stop=(t == nt - 1),
                                    tile_position=(0, q * W))
                        # epilogue: h = relu(dis*agg + bias)
                        htmp = wk.tile([P, Fh], F32, tag="htmp")
                        nc.vector.tensor_scalar_mul(htmp[:], agg[:],
                                                    t_disb[:, b:b + 1])
                        nc.vector.tensor_tensor(htmp[:], htmp[:], postbias[:],
                                                op=mybir.AluOpType.add)
                        h = wk.tile([P, Fh], BF, tag="hblk")
                        nc.scalar.activation(h[:], htmp[:],
                                             mybir.ActivationFunctionType.Relu)
                        if t_wnext is not None:
                            tp = psT.tile([P, P], BF, tag="tps")
                            nc.tensor.transpose(tp[0:Fh, :], h[:], t_idn[:])
                            hT = wk.tile([P, P], BF, tag="hT")
                            nc.vector.tensor_copy(hT[0:Fh, :], tp[0:Fh, :])
                            zp = psZ.tile([P, Fo], F32, tag="zps")
                            nc.tensor.matmul(zp[:], lhsT=hT[0:Fh, :],
                                             rhs=t_wnext[:], start=True,
                                             stop=True)
                            nc.vector.tensor_scalar_mul(
                                zs_buf[l + 1][:, b * Fo:(b + 1) * Fo], zp[:],
                                t_disb[:, b:b + 1])
                        else:
                            classifier(b, h)
                if t_wnext is not None:
                    nc.sync.dma_start(
                        z_local[l + 1][:].rearrange(
                            "(j p) s -> p j s", p=P)[:, :, 0:Fo],
                        zs_buf[l + 1][:].rearrange("p (j h) -> p j h", h=Fo))

            def classifier(b, h4):
                # z4 = x4 @ fc1W + fc1b ; r = relu(LN(z4)) ; out = r@fc2W + fc2b
                tp = psT.tile([P, P], BF, tag="tps")
                nc.tensor.transpose(tp[0:H3, :], h4[:], t_idn[:])
                hT = wk.tile([P, P], BF, tag="hT")
                nc.vector.tensor_copy(hT[0:H3, :], tp[0:H3, :])
                zp = psZ.tile([P, HC], F32, tag="zps")
                nc.tensor.matmul(zp[:], lhsT=hT[0:H3, :], rhs=t_fc1w[:],
                                 start=True, stop=True)
                z4 = wk.tile([P, HC], F32, tag="z4")
                nc.vector.tensor_tensor(z4[:], zp[:], t_rep["fc1b"][:],
                                        op=mybir.AluOpType.add)
                ssum = sm.tile([P, 1], F32, tag="ssum")
                nc.vector.reduce_sum(ssum[:], z4[:], axis=mybir.AxisListType.X,
                                     negate=True)
                negmean = sm.tile([P, 1], F32, tag="negmean")
                nc.vector.tensor_scalar_mul(negmean[:], ssum[:], 1.0 / HC)
                sq = wk.tile([P, HC], F32, tag="sq4")
                sqs = sm.tile([P, 1], F32, tag="sqs")
                nc.scalar.activation(sq[:], z4[:],
                                     mybir.ActivationFunctionType.Square,
                                     bias=negmean[:], scale=1.0,
                                     accum_out=sqs[:])
                std = sm.tile([P, 1], F32, tag="std")
                nc.scalar.activation(std[:], sqs[:],
                                     mybir.ActivationFunctionType.Sqrt,
                                     bias=t_eps[:], scale=1.0 / HC)
                rstd = sm.tile([P, 1], F32, tag="rstd")
                nc.vector.reciprocal(rstd[:], std[:])
                nmr = sm.tile([P, 1], F32, tag="nmr")
                nc.vector.tensor_tensor(nmr[:], negmean[:], rstd[:],
                                        op=mybir.AluOpType.mult)
                xln = wk.tile([P, HC], F32, tag="xln4")
                nc.vector.tensor_scalar(xln[:], z4[:], rstd[:], nmr[:],
                                        op0=mybir.AluOpType.mult,
                                        op1=mybir.AluOpType.add)
                nc.vector.tensor_tensor(xln[:], xln[:], t_rep["lncg"][:],
                                        op=mybir.AluOpType.mult)
                nc.vector.tensor_tensor(xln[:], xln[:], t_rep["lncb"][:],
                                        op=mybir.AluOpType.add)
                r4 = wk.tile([P, HC], BF, tag="r4")
                nc.scalar.activation(r4[:], xln[:],
                                     mybir.ActivationFunctionType.Relu)
                tp2 = psT.tile([P, P], BF, tag="tps")
                nc.tensor.transpose(tp2[0:HC, :], r4[:], t_idn[:])
                rT = wk.tile([P, P], BF, tag="rT")
                nc.vector.tensor_copy(rT[0:HC, :], tp2[0:HC, :])
                op2 = psZ.tile([P, C], F32, tag="zps")
                nc.tensor.matmul(op2[:], lhsT=rT[0:HC, :], rhs=t_fc2w[:],
                                 start=True, stop=True)
                nc.vector.tensor_tensor(out_buf[:, b * C:(b + 1) * C], op2[:],
                                        t_rep["fc2b"][:],
                                        op=mybir.AluOpType.add)

            if stop:
                nc.vector.memset(zs_buf[1][:], 0.0)
                nc.vector.memset(zs_buf[2][:], 0.0)
            if stop == "A":
                nc.vector.memset(out_buf[:], 0.0)
            elif stop in ("AG", "G0", "G1", "L1"):
                edge_layer(0, H1, H2, t_w2, t_rep["b1f"],
                           mode={"AG": "ag", "G0": "gather0", "G1": "gather",
                                 "L1": "full"}[stop])
                nc.vector.memset(out_buf[:], 0.0)
            else:
                edge_layer(0, H1, H2, t_w2, t_rep["b1f"])
                edge_layer(1, H2, H3, t_w3, t_rep["b2f"])
                edge_layer(2, H3, None, None, t_rep["b3f"])

            nc.sync.dma_start(
                ap_out.rearrange("(j p) c -> p j c", p=P),
                out_buf[:].rearrange("p (j c) -> p j c", c=C))
    nc.compile()
    return nc


_CACHE = {}


def _get_nc(cfg):
    key = repr(sorted((k, str(v)) for k, v in cfg.items()))
    if key not in _CACHE:
        _CACHE[key] = build_nc(cfg)
    return _CACHE[key]


def kernel(**inputs):
    cfg, in_maps = preprocess(**inputs)
    nc = _get_nc(cfg)
    res = bass_utils.run_bass_kernel_spmd(nc, in_maps, core_ids=list(range(NC)))
    NPC, NPAD, N, C = cfg["NPC"], cfg["NPAD"], cfg["N"], cfg["C"]
    out = np.empty((N, C), np.float32)
    for c in range(NC):
        out[c * NPC:(c + 1) * NPC] = res.results[c]["out"][:NPC]
    return out



# revision 3
# speedup vs baseline: 2.6261x; 2.6261x over previous
"""Distributed GCN (3x GCNConv + MLP classifier) on 8 Trainium2 NeuronCores.

Strategy (graph/data parallel, dst-partitioned):
  - nodes are partitioned into 8 contiguous chunks (one per core), padded to a
    multiple of 128; edges are assigned to the core owning their dst node and
    bucketed by (dst block of 128, quarter of 32) with self-loop edges added.
  - per layer: each core computes z = x @ W(folded) for its nodes, scales rows
    by deg_inv_sqrt, AllGathers the z-table (bf16, 256B row stride), then for
    each 128-edge tile dma_gathers z[src] rows and accumulates
    agg[dst] += onehot(dst_rel).T @ z_rows on the TensorEngine (PSUM f32).
  - BatchNorm (eval) and LayerNorm gains are folded into the weights on host.
All heavy math runs on device; the host only preprocesses integer graph
structure, folds parameters, and slices/concatenates per-core arrays.
"""
import sys
import os

for _p in ("/opt/trn_rl_repo",):
    if _p not in sys.path:
        sys.path.insert(0, _p)

import numpy as np
import ml_dtypes

import concourse.bass as bass
import concourse.bacc as bacc
import concourse.tile as tile
import concourse.mybir as mybir
from concourse import bass_utils

BF16 = ml_dtypes.bfloat16
F32 = mybir.dt.float32
BF = mybir.dt.bfloat16
EPS = 1e-5
NC = 8
P = 128          # partitions / block size
W = 32           # one-hot quarter width
TS = 128         # z-table row stride in bf16 elements (256B, dma_gather req)
MAX_GROUP_IDX = 12000   # dma_gather Q7 scratch limit is ~16k int32 idxs


def _patch_dma_gather():
    """Relax bass's elem_size%256B assert: the Q7 ucode only needs the row
    STRIDE to be a multiple of 256B; the payload can be narrower."""
    import inspect, re, textwrap
    import concourse.bass as cbass
    if getattr(cbass.BassGpSimd.dma_gather, "_relaxed_elem_size", False):
        return
    src = textwrap.dedent(inspect.getsource(cbass.BassGpSimd.dma_gather))
    pat = re.compile(
        r"assert \(\s*elem_size_bytes > 0 and elem_size_bytes % 256 == 0\s*\)"
        r"\s*#[^\n]*", re.S)
    assert pat.search(src), "dma_gather source changed; update patch"
    src = pat.sub("assert elem_size_bytes > 0", src)
    ns = vars(cbass).copy()
    exec(compile(src, "<patched_dma_gather>", "exec"), ns)
    ns["dma_gather"]._relaxed_elem_size = True
    cbass.BassGpSimd.dma_gather = ns["dma_gather"]


_patch_dma_gather()


def _to_bf(a):
    return np.ascontiguousarray(np.asarray(a, np.float32)).astype(BF16)


def _rep(v, width=None):
    """Replicate a 1-D param across 128 partitions -> [128, len]."""
    v = np.asarray(v, np.float32).reshape(1, -1)
    return np.ascontiguousarray(np.repeat(v, P, 0))


def preprocess(x, edge_index, ln_g, ln_b, W1, b1, bn1_g, bn1_b, bn1_m, bn1_v,
               W2, b2, bn2_g, bn2_b, bn2_m, bn2_v, W3, b3, bn3_g, bn3_b, bn3_m,
               bn3_v, fc1_W, fc1_b, lnc_g, lnc_b, fc2_W, fc2_b):
    N, D = x.shape
    E = edge_index.shape[1]
    H1, H2, H3 = W1.shape[1], W2.shape[1], W3.shape[1]
    HC, C = fc1_W.shape[1], fc2_W.shape[1]
    assert N % NC == 0, N
    NPC = N // NC
    NBLK = (NPC + P - 1) // P
    NPAD = NBLK * P
    NTAB = NC * NPAD
    assert NTAB <= 65536
    BIAS = max(0, NTAB - 32768)

    src = np.asarray(edge_index[0], np.int64)
    dst = np.asarray(edge_index[1], np.int64)
    deg = np.bincount(dst, minlength=N).astype(np.float32) + 1.0
    dis = 1.0 / np.sqrt(deg)

    # fold LN gain + BN(eval) into weights; biases:
    #   z1 = x_ln_raw @ W1f + zb1 (per-row const), post-agg bias b1f
    k1 = bn1_g / np.sqrt(bn1_v + EPS)
    W1f = (np.asarray(ln_g)[:, None] * np.asarray(W1)) * k1[None, :]
    zb1 = (np.asarray(ln_b) @ np.asarray(W1)) * k1
    b1f = np.asarray(b1) * k1 + (bn1_b - bn1_m * k1)
    k2 = bn2_g / np.sqrt(bn2_v + EPS)
    W2f = np.asarray(W2) * k2[None, :]
    b2f = np.asarray(b2) * k2 + (bn2_b - bn2_m * k2)
    k3 = bn3_g / np.sqrt(bn3_v + EPS)
    W3f = np.asarray(W3) * k3[None, :]
    b3f = np.asarray(b3) * k3 + (bn3_b - bn3_m * k3)

    # edge lists: original edges + self loops, assigned to dst owner core
    src_all = np.concatenate([src, np.arange(N, dtype=np.int64)])
    dst_all = np.concatenate([dst, np.arange(N, dtype=np.int64)])
    core_of = dst_all // NPC
    dloc = dst_all - core_of * NPC
    srcpad = (src_all // NPC) * NPAD + (src_all % NPC)

    ncell = NBLK * 4
    counts = np.zeros((NC, ncell), np.int64)
    per_core = []
    for c in range(NC):
        m = core_of == c
        s = srcpad[m]
        d = dloc[m]
        o = np.argsort(d, kind="stable")
        s, d = s[o], d[o]
        cell = (d >> 5).astype(np.int64)  # block*4 + quarter
        counts[c] = np.bincount(cell, minlength=ncell)
        per_core.append((s, d, cell))

    T = np.maximum(1, -(-counts.max(0) // P))      # tiles per cell, shared
    tile_off = np.concatenate([[0], np.cumsum(T)]).astype(np.int64)
    ntiles = int(tile_off[-1])

    idx16_list, dstrel_list = [], []
    for c in range(NC):
        s, d, cell = per_core[c]
        start = np.searchsorted(cell, np.arange(ncell))
        pos = np.arange(len(cell)) - start[cell]
        slot = tile_off[cell] * P + pos
        idx_lin = np.zeros(ntiles * P, np.int32)          # pad -> row BIAS
        rel_lin = np.full(ntiles * P, 99.0, np.float32)   # pad -> no match
        idx_lin[slot] = (s - BIAS).astype(np.int32)
        rel_lin[slot] = (d & 31).astype(np.float32)
        assert idx_lin.min() >= -32768 and idx_lin.max() <= 32767
        idx16 = idx_lin.reshape(ntiles * 8, 16).T.astype(np.int16)  # n=(n%16,n//16)
        idx16 = np.tile(idx16, (8, 1))                     # replicate: [128, ntiles*8]
        dstrel = rel_lin.reshape(ntiles, P).T              # [128, ntiles]
        idx16_list.append(np.ascontiguousarray(idx16))
        dstrel_list.append(np.ascontiguousarray(_to_bf(dstrel)))

    # group blocks so one dma_gather stays under the Q7 scratch limit
    groups = []  # list of (b0, nb) block ranges
    b0 = 0
    while b0 < NBLK:
        nb = 0
        while (b0 + nb < NBLK and nb < 8
               and (tile_off[(b0 + nb + 1) * 4] - tile_off[b0 * 4]) * P
               <= MAX_GROUP_IDX):
            nb += 1
        nb = max(nb, 1)
        groups.append((b0, nb))
        b0 += nb

    # per-core node data
    xp_list, disb_list = [], []
    dis_pad = np.ones(NC * NPAD, np.float32)
    for c in range(NC):
        xp = np.zeros((NPAD, D), np.float32)
        xp[:NPC] = np.asarray(x[c * NPC:(c + 1) * NPC], np.float32)
        xp_list.append(xp)
        dis_pad[c * NPAD:c * NPAD + NPC] = dis[c * NPC:(c + 1) * NPC]
        disb = dis_pad[c * NPAD:(c + 1) * NPAD].reshape(NBLK, P).T
        disb_list.append(np.ascontiguousarray(disb))

    iota = np.tile(np.arange(W, dtype=np.float32), (P, 16))
    ident = np.eye(P, dtype=np.float32)

    consts = dict(
        w1=_to_bf(W1f), w2=_to_bf(W2f), w3=_to_bf(W3f),
        fc1w=_to_bf(np.asarray(fc1_W)), fc2w=_to_bf(np.asarray(fc2_W)),
        zb1=_rep(zb1), b1f=_rep(b1f), b2f=_rep(b2f), b3f=_rep(b3f),
        fc1b=_rep(fc1_b), lncg=_rep(lnc_g), lncb=_rep(lnc_b), fc2b=_rep(fc2_b),
        iota=_to_bf(iota), idn=_to_bf(ident),
    )
    in_maps = []
    for c in range(NC):
        m = dict(consts)
        m.update(xp=xp_list[c], disb=disb_list[c], idx16=idx16_list[c],
                 dstrel=dstrel_list[c])
        in_maps.append(m)

    cfg = dict(N=N, D=D, E=E, H1=H1, H2=H2, H3=H3, HC=HC, C=C, NPC=NPC,
               NBLK=NBLK, NPAD=NPAD, NTAB=NTAB, BIAS=BIAS, ntiles=ntiles,
               T=T.tolist(), tile_off=tile_off.tolist(), groups=groups)
    return cfg, in_maps


def build_nc(cfg):
    stop = cfg.get("stop", "")
    D, H1, H2, H3 = cfg["D"], cfg["H1"], cfg["H2"], cfg["H3"]
    HC, C = cfg["HC"], cfg["C"]
    NBLK, NPAD, NTAB, BIAS = cfg["NBLK"], cfg["NPAD"], cfg["NTAB"], cfg["BIAS"]
    ntiles, T, tile_off = cfg["ntiles"], cfg["T"], cfg["tile_off"]
    groups = cfg["groups"]
    KD = D // P      # k-chunks for layer-1 matmul

    nc = bacc.Bacc("TRN2", target_bir_lowering=False, debug=False,
                   num_devices=NC)
    dt = nc.dram_tensor
    ap_xp = dt("xp", [NPAD, D], F32, kind="ExternalInput").ap()
    ap_disb = dt("disb", [P, NBLK], F32, kind="ExternalInput").ap()
    ap_idx16 = dt("idx16", [P, ntiles * 8], mybir.dt.int16, kind="ExternalInput").ap()
    ap_dstrel = dt("dstrel", [P, ntiles], BF, kind="ExternalInput").ap()
    ap_w1 = dt("w1", [D, H1], BF, kind="ExternalInput").ap()
    ap_w2 = dt("w2", [H1, H2], BF, kind="ExternalInput").ap()
    ap_w3 = dt("w3", [H2, H3], BF, kind="ExternalInput").ap()
    ap_fc1w = dt("fc1w", [H3, HC], BF, kind="ExternalInput").ap()
    ap_fc2w = dt("fc2w", [HC, C], BF, kind="ExternalInput").ap()
    reps = {}
    for nm, wd in [("zb1", H1), ("b1f", H1), ("b2f", H2), ("b3f", H3),
                   ("fc1b", HC), ("lncg", HC), ("lncb", HC), ("fc2b", C)]:
        reps[nm] = dt(nm, [P, wd], F32, kind="ExternalInput").ap()
    ap_iota = dt("iota", [P, 16 * W], BF, kind="ExternalInput").ap()
    ap_idn = dt("idn", [P, P], BF, kind="ExternalInput").ap()
    ap_out = dt("out", [NPAD, C], F32, kind="ExternalOutput").ap()

    with tile.TileContext(nc) as tc:
        with (
            tc.tile_pool(name="const", bufs=1) as cp,
            tc.tile_pool(name="xin", bufs=2) as xin,
            tc.tile_pool(name="work", bufs=2) as wk,
            tc.tile_pool(name="small", bufs=3) as sm,
            tc.tile_pool(name="zbuf", bufs=1) as zb,
            tc.tile_pool(name="gath", bufs=2) as gp,
            tc.tile_pool(name="onehot", bufs=2) as op_,
            tc.tile_pool(name="psA", bufs=2, space="PSUM") as psA,
            tc.tile_pool(name="psZ", bufs=2, space="PSUM") as psZ,
            tc.tile_pool(name="psT", bufs=2, space="PSUM") as psT,
            tc.tile_pool(name="dram", bufs=1, space="DRAM") as dram,
        ):
            # ---- constants to SBUF
            def load_const(ap, shape, dtype):
                t = cp.tile(shape, dtype, tag=f"c{ap.tensor.name}",
                            name=f"c{ap.tensor.name}")
                nc.sync.dma_start(t[:], ap)
                return t

            t_w1 = cp.tile([P, KD * H1], BF, tag="w1")
            nc.sync.dma_start(t_w1[:].rearrange("p (k h) -> p k h", h=H1),
                              ap_w1.rearrange("(k p) h -> p k h", p=P))
            t_w2 = load_const(ap_w2, [H1, H2], BF)
            t_w3 = load_const(ap_w3, [H2, H3], BF)
            t_fc1w = load_const(ap_fc1w, [H3, HC], BF)
            t_fc2w = load_const(ap_fc2w, [HC, C], BF)
            t_rep = {}
            for nm in reps:
                t_rep[nm] = load_const(reps[nm], list(reps[nm].shape), F32)
            t_iota = load_const(ap_iota, [P, 16 * W], BF)
            t_idn = load_const(ap_idn, [P, P], BF)
            t_disb = load_const(ap_disb, [P, NBLK], F32)
            t_eps = cp.tile([P, 1], F32, tag="eps")
            nc.vector.memset(t_eps[:], float(EPS))
            t_idx = cp.tile([P, ntiles * 8], mybir.dt.int16, tag="idx")
            nc.sync.dma_start(t_idx[:], ap_idx16)
            t_drel = cp.tile([P, ntiles], BF, tag="drel")
            nc.sync.dma_start(t_drel[:], ap_dstrel)

            z_local = [dram.tile([NPAD, TS], BF, tag=f"zloc{l}",
                                 name=f"zloc{l}") for l in range(3)]
            z_full = [dram.tile([NTAB, TS], BF, tag=f"zfull{l}",
                                name=f"zfull{l}") for l in range(3)]
            HH = [H1, H2, H3]
            zs_buf = [zb.tile([P, NBLK * HH[l]], BF, tag=f"zs{l}",
                              name=f"zs{l}") for l in range(3)]
            out_buf = zb.tile([P, NBLK * C], F32, tag="outb")

            # ============ phase A: LN + z1 per block ============
            for b in range(NBLK):
                xblk = xin.tile([P, D], F32, tag="xblk")
                nc.sync.dma_start(xblk[:], ap_xp[b * P:(b + 1) * P, :])
                ssum = sm.tile([P, 1], F32, tag="ssum")
                nc.vector.reduce_sum(ssum[:], xblk[:], axis=mybir.AxisListType.X,
                                     negate=True)
                negmean = sm.tile([P, 1], F32, tag="negmean")
                nc.vector.tensor_scalar_mul(negmean[:], ssum[:], 1.0 / D)
                sq = wk.tile([P, D], F32, tag="sq")
                sqs = sm.tile([P, 1], F32, tag="sqs")
                nc.scalar.activation(sq[:], xblk[:],
                                     mybir.ActivationFunctionType.Square,
                                     bias=negmean[:], scale=1.0,
                                     accum_out=sqs[:])
                std = sm.tile([P, 1], F32, tag="std")
                nc.scalar.activation(std[:], sqs[:],
                                     mybir.ActivationFunctionType.Sqrt,
                                     bias=t_eps[:], scale=1.0 / D)
                rstd = sm.tile([P, 1], F32, tag="rstd")
                nc.vector.reciprocal(rstd[:], std[:])
                nmr = sm.tile([P, 1], F32, tag="nmr")
                nc.vector.tensor_tensor(nmr[:], negmean[:], rstd[:],
                                        op=mybir.AluOpType.mult)
                xln = wk.tile([P, D], BF, tag="xln")
                nc.vector.tensor_scalar(xln[:], xblk[:], rstd[:], nmr[:],
                                        op0=mybir.AluOpType.mult,
                                        op1=mybir.AluOpType.add)
                # transpose to [D, 128] (two 128-chunks), then z1
                zp = psZ.tile([P, H1], F32, tag="zps")
                for kc in range(KD):
                    tp = psT.tile([P, P], BF, tag="tps")
                    nc.tensor.transpose(tp[:], xln[:, kc * P:(kc + 1) * P],
                                        t_idn[:])
                    xT = wk.tile([P, P], BF, tag="xT")
                    nc.vector.tensor_copy(xT[:], tp[:])
                    nc.tensor.matmul(zp[:], lhsT=xT[:],
                                     rhs=t_w1[:, kc * H1:(kc + 1) * H1],
                                     start=(kc == 0), stop=(kc == KD - 1))
                ztmp = wk.tile([P, H1], F32, tag="ztmp")
                nc.vector.tensor_tensor(ztmp[:], zp[:], t_rep["zb1"][:],
                                        op=mybir.AluOpType.add)
                nc.vector.tensor_scalar_mul(
                    zs_buf[0][:, b * H1:(b + 1) * H1], ztmp[:],
                    t_disb[:, b:b + 1])
            nc.sync.dma_start(
                z_local[0][:].rearrange("(j p) s -> p j s", p=P)[:, :, 0:H1],
                zs_buf[0][:].rearrange("p (j h) -> p j h", h=H1))

            # ============ per-layer edge phases ============
            def edge_layer(l, Fh, Fo, t_wnext, postbias, mode="full"):
                """layer l: gather z_l, aggregate, epilogue -> h; z_{l+1} or
                classifier input written to zs_buf[l+1] (if t_wnext) else
                returns h tiles via classifier()."""
                if cfg.get("no_cc"):
                    # timeline-sim proxy: collectives replaced by equivalent
                    # local DMA traffic (single-core TimelineSim only)
                    for c in range(NC):
                        nc.sync.dma_start(
                            z_full[l][c * NPAD:(c + 1) * NPAD, :], z_local[l][:])
                else:
                    nc.gpsimd.collective_compute(
                        "AllGather", mybir.AluOpType.bypass,
                        replica_groups=[list(range(NC))],
                        ins=[z_local[l][:].opt()], outs=[z_full[l][:].opt()],
                    )
                if mode == "ag":
                    return
                only_gather = mode in ("gather", "gather0")
                for (b0, nb) in groups:
                    t0 = tile_off[b0 * 4]
                    t1 = tile_off[(b0 + nb) * 4]
                    gt = t1 - t0
                    gbuf = gp.tile([P, gt * Fh], BF, tag="gbuf")
                    nc.gpsimd.dma_gather(
                        out_ap=gbuf[:].rearrange("p (n f) -> p n f", f=Fh),
                        in_ap=z_full[l][BIAS:, 0:Fh],
                        idxs_ap=t_idx[:, t0 * 8:t1 * 8],
                        num_idxs=gt * P,
                        num_idxs_reg=gt * P,
                        elem_size=Fh,
                        elem_step=TS,
                        single_packet=False,
                    )
                    sbuf = op_.tile([P, gt * W], BF, tag="sbufS")
                    if mode == "gather0":
                        nc.vector.tensor_copy(out_buf[:, 0:C], gbuf[:, 0:C])
                        continue
                    for s0 in range(0, gt, 16):
                        s1 = min(s0 + 16, gt)
                        dr = t_drel[:, t0 + s0:t0 + s1]
                        dr_b = bass.AP(dr.tensor, dr.offset, dr.ap + [[0, W]])
                        nc.vector.tensor_tensor(
                            out=sbuf[:, s0 * W:s1 * W].rearrange(
                                "p (t w) -> p t w", w=W),
                            in0=t_iota[:, 0:(s1 - s0) * W].rearrange(
                                "p (t w) -> p t w", w=W),
                            in1=dr_b,
                            op=mybir.AluOpType.is_equal)
                    if mode == "gather":
                        nc.vector.tensor_copy(out_buf[:, 0:C], gbuf[:, 0:C])
                        nc.vector.tensor_copy(out_buf[:, C:2 * C],
                                              sbuf[:, 0:C])
                        continue
                    for b in range(b0, b0 + nb):
                        agg = psA.tile([P, Fh], F32, tag="agg")
                        for q in range(4):
                            cell = b * 4 + q
                            nt = T[cell]
                            base = tile_off[cell]
                            for t in range(nt):
                                g = base + t - t0
                                nc.tensor.matmul(
                                    agg[q * W:(q + 1) * W, :],
                                    lhsT=sbuf[:, g * W:(g + 1) * W],
                                    rhs=gbuf[:, g * Fh:(g + 1) * Fh],
                                    start=(t == 0), stop=(t == nt - 1),
                                    tile_position=(0, q * W))
                        # epilogue: h = relu(dis*agg + bias)
                        htmp = wk.tile([P, Fh], F32, tag="htmp")
                        nc.vector.tensor_scalar_mul(htmp[:], agg[:],
                                                    t_disb[:, b:b + 1])
                        nc.vector.tensor_tensor(htmp[:], htmp[:], postbias[:],
                                                op=mybir.AluOpType.add)
                        h = wk.tile([P, Fh], BF, tag="hblk")
                        nc.scalar.activation(h[:], htmp[:],
                                             mybir.ActivationFunctionType.Relu)
                        if t_wnext is not None:
                            tp = psT.tile([P, P], BF, tag="tps")
                            nc.tensor.transpose(tp[0:Fh, :], h[:], t_idn[:])
                            hT = wk.tile([P, P], BF, tag="hT")
                            nc.vector.tensor_copy(hT[0:Fh, :], tp[0:Fh, :])
                            zp = psZ.tile([P, Fo], F32, tag="zps")
                            nc.tensor.matmul(zp[:], lhsT=hT[0:Fh, :],
                                             rhs=t_wnext[:], start=True,
                                             stop=True)
                            nc.vector.tensor_scalar_mul(
                                zs_buf[l + 1][:, b * Fo:(b + 1) * Fo], zp[:],
                                t_disb[:, b:b + 1])
                        else:
                            classifier(b, h)
                if t_wnext is not None:
                    nc.sync.dma_start(
                        z_local[l + 1][:].rearrange(
                            "(j p) s -> p j s", p=P)[:, :, 0:Fo],
                        zs_buf[l + 1][:].rearrange("p (j h) -> p j h", h=Fo))

            def classifier(b, h4):
                # z4 = x4 @ fc1W + fc1b ; r = relu(LN(z4)) ; out = r@fc2W + fc2b
                tp = psT.tile([P, P], BF, tag="tps")
                nc.tensor.transpose(tp[0:H3, :], h4[:], t_idn[:])
                hT = wk.tile([P, P], BF, tag="hT")
                nc.vector.tensor_copy(hT[0:H3, :], tp[0:H3, :])
                zp = psZ.tile([P, HC], F32, tag="zps")
                nc.tensor.matmul(zp[:], lhsT=hT[0:H3, :], rhs=t_fc1w[:],
                                 start=True, stop=True)
                z4 = wk.tile([P, HC], F32, tag="z4")
                nc.vector.tensor_tensor(z4[:], zp[:], t_rep["fc1b"][:],
                                        op=mybir.AluOpType.add)
                ssum = sm.tile([P, 1], F32, tag="ssum")
                nc.vector.reduce_sum(ssum[:], z4[:], axis=mybir.AxisListType.X,
                                     negate=True)
                negmean = sm.tile([P, 1], F32, tag="negmean")
                nc.vector.tensor_scalar_mul(negmean[:], ssum[:], 1.0 / HC)
                sq = wk.tile([P, HC], F32, tag="sq4")
                sqs = sm.tile([P, 1], F32, tag="sqs")
                nc.scalar.activation(sq[:], z4[:],
                                     mybir.ActivationFunctionType.Square,
                                     bias=negmean[:], scale=1.0,
                                     accum_out=sqs[:])
                std = sm.tile([P, 1], F32, tag="std")
                nc.scalar.activation(std[:], sqs[:],
                                     mybir.ActivationFunctionType.Sqrt,
                                     bias=t_eps[:], scale=1.0 / HC)
                rstd = sm.tile([P, 1], F32, tag="rstd")
                nc.vector.reciprocal(rstd[:], std[:])
                nmr = sm.tile([P, 1], F32, tag="nmr")
                nc.vector.tensor_tensor(nmr[:], negmean[:], rstd[:],
                                        op=mybir.AluOpType.mult)
                xln = wk.tile([P, HC], F32, tag="xln4")
                nc.vector.tensor_scalar(xln[:], z4[:], rstd[:], nmr[:],
                                        op0=mybir.AluOpType.mult,
                                        op1=mybir.AluOpType.add)
                nc.vector.tensor_tensor(xln[:], xln[:], t_rep["lncg"][:],
                                        op=mybir.AluOpType.mult)
                nc.vector.tensor_tensor(xln[:], xln[:], t_rep["lncb"][:],
                                        op=mybir.AluOpType.add)
                r4 = wk.tile([P, HC], BF, tag="r4")
                nc.scalar.activation(r4[:], xln[:],
                                     mybir.ActivationFunctionType.Relu)
                tp2 = psT.tile([P, P], BF, tag="tps")
                nc.tensor.transpose(tp2[0:HC, :], r4[:], t_idn[:])
                rT = wk.tile([P, P], BF, tag="rT")
                nc.vector.tensor_copy(rT[0:HC, :], tp2[0:HC, :])
                op2 = psZ.tile([P, C], F32, tag="zps")
                nc.tensor.matmul(op2[:], lhsT=rT[0:HC, :], rhs=t_fc2w[:],
                                 start=True, stop=True)
                nc.vector.tensor_tensor(out_buf[:, b * C:(b + 1) * C], op2[:],
                                        t_rep["fc2b"][:],
                                        op=mybir.AluOpType.add)

            if stop:
                nc.vector.memset(zs_buf[1][:], 0.0)
                nc.vector.memset(zs_buf[2][:], 0.0)
            if stop == "A":
                nc.vector.memset(out_buf[:], 0.0)
            elif stop in ("AG", "G0", "G1", "L1"):
                edge_layer(0, H1, H2, t_w2, t_rep["b1f"],
                           mode={"AG": "ag", "G0": "gather0", "G1": "gather",
                                 "L1": "full"}[stop])
                nc.vector.memset(out_buf[:], 0.0)
            else:
                edge_layer(0, H1, H2, t_w2, t_rep["b1f"])
                edge_layer(1, H2, H3, t_w3, t_rep["b2f"])
                edge_layer(2, H3, None, None, t_rep["b3f"])

            nc.sync.dma_start(
                ap_out.rearrange("(j p) c -> p j c", p=P),
                out_buf[:].rearrange("p (j c) -> p j c", c=C))
    nc.compile()
    return nc


_CACHE = {}


def _get_nc(cfg):
    key = repr(sorted((k, str(v)) for k, v in cfg.items()))
    if key not in _CACHE:
        _CACHE[key] = build_nc(cfg)
    return _CACHE[key]


def kernel(**inputs):
    cfg, in_maps = preprocess(**inputs)
    nc = _get_nc(cfg)
    res = bass_utils.run_bass_kernel_spmd(nc, in_maps, core_ids=list(range(NC)))
    NPC, NPAD, N, C = cfg["NPC"], cfg["NPAD"], cfg["N"], cfg["C"]
    out = np.empty((N, C), np.float32)
    for c in range(NC):
        out[c * NPC:(c + 1) * NPC] = res.results[c]["out"][:NPC]
    return out

